# revision 36
# baseline (speedup 1.0000x reference)
"""FISTA sparse-coding encoder kernel for Trainium2 (8 NeuronCores).

Problem: x [2,10,20480] f32, Drr/Dtheta [40] f32.
  D = normalized dictionary [10, 161]
  A = I - D^T D / L,  DtY = D^T Y / L,  lam = gamma / L
  40 FISTA iterations: xn = softshrink(A @ y + DtY); y = xn + m (xn - x_old)
  output sparsecode [2, 161, 20480].

Design ("u-form", v4: fp16 matmuls + PSUM-packed tails):
  - Data-parallel over columns: Y reshaped to [10, 40960]; 5120 columns/core.
  - Momentum identity: A y_i + DtY = (1+m) u_i - m u_{i-1} with
    u_i = A x_i + DtY, so each iteration needs one matmul pass over x_i and
    one fused elementwise op  xn = shrink(s0*u_i - m*u_{i-1}, lam).
  - Output rows split 118 (head) + 43 (tail).  Contraction split 128 + 43:
    block 1 = [x rows 0..117 ; Y glue rows] (DtY via the glue), block 2 =
    x rows 118..160 packed at partition bases 0/64 of xT.
  - Tail packing: the two 512-column halves of a 1024-column group write
    their 43 tail rows into ONE [107, 512] PSUM tile at partition offsets 0
    and 64 (PE quad-tile col positions; half A's tail out-block is 64 wide
    with 21 zero weight columns so the pad partitions are initialized).
    The tail DVE/ACT ops then process 1024 columns in 512 free-cycles,
    cutting elementwise busy/iter from 2N to 1.5N.  PSUM partition offsets
    require 16-bit matmul operands (walrus rejects f32r quad-tiling), so
    x-state and weights are fp16: u stays f32 in PSUM, the momentum
    combines f32 u's, and only x is rounded; measured end-to-end error
    ~9e-3 vs the f32 reference (tolerance 2e-2).
  - Per iteration per group:
      PE:  8 matmuls -> u head [118,1024] + u tail packed [107,512] (f32)
      ACT: evacuate raw u -> esH/esT[it%2] (doubles as next iteration's
           u_{i-1}; all-SBUF reads are cheaper for the DVE than PSUM)
      DVE: fused shrink+momentum  xn = shrink(s0*es_cur + s1*es_prev)
           -> fp16 x state (f32 tiles on the last iteration for the DMA)
  - Warm-up: dummy matmuls ramp the PE p-state before real work dispatches;
    a no-op activation preloads the ACT function table; init is chunked by
    column group so the pipeline starts as soon as group 0 is staged.
"""

import numpy as np

# ---------------------------------------------------------------- constants
B, T, N_POLES, P = 2, 10, 40, 20480
MAX_ITER = 40
GAMMA = 0.01
K = 4 * N_POLES + 1          # 161
NCORES = 8
NCOLS = B * P // NCORES      # 5120 columns per core
GRP = 1024                   # column group (head psum = 2 banks)
HB = 512                     # half-block (one PSUM bank, matmul free dim)
NGRP = NCOLS // GRP          # 5
KH = 118                     # head out rows (x rows 0..117)
KT = K - KH                  # 43 tail out rows (x rows 118..160)
PCOLS = NCOLS // 2           # 2560 packed tail columns
TB = 64                      # packed-tail half B base partition
XT_P = TB + KT               # 107 partitions of packed tail state

_cache = {}


# ------------------------------------------------------------- custom DVE op
def _register_shrinkmom3():
    """out = relu(w - C2) + min(w + C2, 0)  with  w = in0*s0 + in1*s1.

    Softshrink of an affine combination of two raw tensors:
    s0 = (1+m_prev), s1 = -m_prev, imm2 = lam.  8 ALU stages.
    """
    import concourse.dve_ops as dve_ops
    from concourse.dve_spec import (
        Spec, Src0, Src1, C0, C1, C2, Zero, relu, minn, lower,
    )
    from concourse.dve_spec import _has_src1 as has_src1
    from concourse.dve_uop import DveOpSpec

    name = "ANT_SHRINKMOM3_FISTA"
    if any(op.name == name for op in dve_ops.OPS):
        return next(op for op in dve_ops.OPS if op.name == name)

    w = Src0 * C0 + Src1 * C1
    spec = Spec(
        body=relu(w - C2) + minn(w + C2, Zero),
        reference=lambda in0, in1, s0=1.0, s1=0.0, imm2=0.0: (
            lambda ww: (np.maximum(ww - imm2, 0.0)
                        + np.minimum(ww + imm2, 0.0)).astype(np.float32)
        )(in0 * s0 + in1 * s1),
    )
    op = dve_ops.DveOp(name, spec, subdim=False, uops_sha={})
    dve_ops.OPS.append(op)
    dve_ops.CUSTOM_DVE_SPECS[name] = spec
    dve_ops._SUB_OPCODE_FOR_NAME[name] = (
        dve_ops._CUSTOM_DVE_ROW_BASE + len(dve_ops.OPS) - 1
    )
    for ver in ("v3", "v4"):
        compiled = DveOpSpec(
            name=name,
            opcode=dve_ops.get_dve_sub_opcode(name),
            uops=lower(spec, ver=ver),
            rd1_en=has_src1(spec),
        )
        op.uops_sha[ver] = compiled.sha(ver)
    return op


# ------------------------------------------------------------ host constants
def _host_constants(Drr, Dtheta):
    r = Drr.astype(np.float64)
    th = Dtheta.astype(np.float64)
    i = np.arange(T, dtype=np.float64)[:, None]
    pr = r[None, :] ** i
    sgn = np.where(np.arange(T)[:, None] % 2 == 0, 1.0, -1.0)
    c = np.cos(i * th[None, :])
    s = np.sin(i * th[None, :])
    ones = np.ones((T, 1))
    dic = np.concatenate([ones, pr * c, sgn * pr * c, pr * s, sgn * pr * s], axis=1)
    G = np.linalg.norm(dic, axis=0)
    G = np.where(G == 0, np.sqrt(float(T)), G)
    D = (dic / G).astype(np.float32)            # [T, K]

    D64 = D.astype(np.float64)
    DtD = D64.T @ D64
    L = float(np.linalg.norm(DtD))              # Frobenius
    A = np.eye(K) - DtD / L                     # [K, K] (symmetric)
    lam = float(GAMMA / L)

    # contraction block 1 rows: [A rows 0..117 ; D/L glue rows]  [128, K]
    w1 = np.concatenate([A[0:KH, :], D64 / L], axis=0).astype(np.float32)
    # contraction block 2 rows: A rows 118..160  [43, K]
    w2 = A[KH:K, :].astype(np.float32)

    # momentum coefficients m_i = (t_i - 1)/t_{i+1}, t_0 = 1
    ms = []
    t = 1.0
    for _ in range(MAX_ITER):
        t_new = (1.0 + np.sqrt(1.0 + 4.0 * t * t)) / 2.0
        ms.append((t - 1.0) / t_new)
        t = t_new
    return w1, w2, lam, ms


# ------------------------------------------------------------- bass program
def _build_program():
    import concourse.mybir as mybir
    import concourse.tile as tile
    from concourse import bacc

    fused_op = _register_shrinkmom3()

    f32 = mybir.dt.float32
    f16 = mybir.dt.float16

    nc = bacc.Bacc("TRN2", target_bir_lowering=False, debug=False,
                   num_devices=NCORES)

    ycols = nc.dram_tensor("ycols", [T, NCOLS], f32, kind="ExternalInput")
    # all weights in one tensor (one DMA): cols 0:118 = l1a, 118:182 = l1b,
    # 182:300 = l2a (rows 0:43), 300:364 = l2b (rows 0:43).  The tail-out
    # weights (l1b/l2b) carry 21 zero columns so the matmuls also zero-fill
    # the pad partitions 43..63 of the packed PSUM tile (free — matmul cost
    # depends only on the moving free size).
    WCOLS = 2 * (KH + TB)
    d_wts = nc.dram_tensor("wts", [128, WCOLS], f32, kind="ExternalInput")
    out = nc.dram_tensor("out", [K, NCOLS], f32, kind="ExternalOutput")

    lam, ms = _cache["consts_meta"]

    with tile.TileContext(nc) as tc:
        with (
            tc.tile_pool(name="state", bufs=1) as st,
            tc.tile_pool(name="wts", bufs=1) as wts,
            tc.tile_pool(name="psH", bufs=2, space="PSUM") as psH,
            tc.tile_pool(name="psT", bufs=3, space="PSUM") as psT,
            tc.tile_pool(name="psW", bufs=1, space="PSUM") as psW,
        ):
            # ---- persistent state -------------------------------------
            # xH rows 0..117 = x head; rows 118..127 = Y glue (written once)
            xH = [st.tile([128, NCOLS], f16, tag=f"xH{b}", name=f"xH{b}")
                  for b in range(2)]
            # packed tail: half A at partitions 0..42, half B at 64..106
            xT = [st.tile([XT_P, PCOLS], f16, tag=f"xT{b}", name=f"xT{b}")
                  for b in range(2)]
            # raw u evacuations, double-buffered: esX[it%2] holds u_it; the
            # fused op reads the current one as in0 (all-SBUF is cheaper
            # for the DVE than PSUM) and the previous one as in1
            esH = [st.tile([KH, NCOLS], f32, tag=f"esH{b}", name=f"esH{b}")
                   for b in range(2)]
            esT = [st.tile([XT_P, PCOLS], f32, tag=f"esT{b}", name=f"esT{b}")
                   for b in range(2)]
            # f32 copies of the final iterate for the output DMA
            foH = st.tile([KH, NCOLS], f32, tag="foH", name="foH")
            foT = st.tile([XT_P, PCOLS], f32, tag="foT", name="foT")

            # fp32 staging for DMA'd weights -> fp16 copies (on DVE, which
            # is idle during init; ACT is busy with the xH[0] chunks)
            ws = wts.tile([128, WCOLS], f32, tag="ws", name="ws")
            l1a = wts.tile([128, KH], f16, tag="l1a", name="l1a")
            l1b = wts.tile([128, TB], f16, tag="l1b", name="l1b")
            # block-2 weights duplicated at partition bases 0 and 64
            l2a = wts.tile([XT_P, KH], f16, tag="l2a", name="l2a")
            l2b = wts.tile([XT_P, TB], f16, tag="l2b", name="l2b")

            # ACT warm-up: a no-op activation so the one-time activation
            # table load (1.3us) happens immediately instead of on the
            # critical first xH copy
            tiny = wts.tile([8, 16], f32, tag="tiny", name="tiny")
            nc.gpsimd.memset(tiny[:], 0.0)
            nc.scalar.mul(tiny[:], tiny[:], 1.0)

            # PE warm-up: the cost model prices matmuls at dispatch time by
            # how long the PE has been continuously busy (p-state ramp), so
            # a short chain of dummy matmuls on an early-ready zero tile
            # ramps the clock before the real work dispatches
            warm = wts.tile([64, HB], f16, tag="warm", name="warm")
            wps = psW.tile([64, HB], mybir.dt.float32, tag="wps", name="wps")
            nc.gpsimd.memset(warm[:], 0.0)
            for i in range(7):
                nc.tensor.matmul(wps[:], warm[:, 0:64], warm[:],
                                 start=(i == 0), stop=(i == 6),
                                 skip_group_check=True)

            nc.sync.dma_start(ws[:], d_wts[:])
            W2 = KH + TB
            nc.vector.tensor_copy(l1a[:], ws[:, 0:KH])
            nc.vector.tensor_copy(l1b[:], ws[:, KH:W2])
            nc.vector.tensor_copy(l2a[0:KT, :], ws[0:KT, W2:W2 + KH])
            nc.vector.tensor_copy(l2b[0:KT, :], ws[0:KT, W2 + KH:WCOLS])
            nc.vector.tensor_copy(l2a[TB:TB + KT, :], ws[0:KT, W2:W2 + KH])
            nc.vector.tensor_copy(l2b[TB:TB + KT, :], ws[0:KT, W2 + KH:WCOLS])

            # ---- init ------------------------------------------------
            # x_0 = 0 (so iteration 0 computes u_0 = DtY from the glue
            # alone).  Zeros + glue-Y staged in f32 and engine-copied
            # (converting) into the fp16 state.  Everything is chunked by
            # column group so iteration 0's first matmuls start as soon as
            # group 0's state is ready.  Iteration 0's fused ops read their
            # (x0-scaled, s1=0) u_{i-1} term from the zeroed zst staging,
            # so the es tiles need no init at all.
            zst = wts.tile([128, NCOLS], f32, tag="zst", name="zst")
            # 3 input DMAs: a small first chunk so group 0 unblocks fast
            ysl = [slice(0, GRP), slice(GRP, 3 * GRP), slice(3 * GRP, NCOLS)]
            for g in range(NGRP):
                gs = slice(g * GRP, (g + 1) * GRP)
                pc = slice(g * HB, (g + 1) * HB)
                nc.gpsimd.memset(zst[0:KH, gs], 0.0)
                if g == 0:
                    nc.sync.dma_start(zst[KH:128, ysl[0]], ycols[:, ysl[0]])
                elif g == 1:
                    nc.sync.dma_start(zst[KH:128, ysl[1]], ycols[:, ysl[1]])
                elif g == 3:
                    nc.sync.dma_start(zst[KH:128, ysl[2]], ycols[:, ysl[2]])
                if g == 0:
                    # split so half A's matmuls unblock ~0.6us earlier
                    nc.scalar.copy(xH[0][:, 0:HB], zst[:, 0:HB])
                    nc.scalar.copy(xH[0][:, HB:GRP], zst[:, HB:GRP])
                else:
                    nc.scalar.copy(xH[0][:, gs], zst[:, gs])
                nc.gpsimd.tensor_copy(xT[0][:, pc],
                                      zst[0:XT_P, g * HB:(g + 1) * HB])
                # xH[1] only needs its glue rows before iteration 1 reads
                # them (rows 0..117 are written by iteration 0's DVE)
                nc.gpsimd.tensor_copy(xH[1][96:128, gs], zst[96:128, gs])

            def mm(ps, lhsT, rhs, start, stop):
                nc.tensor.matmul(ps, lhsT, rhs, start=start, stop=stop,
                                 skip_group_check=True)

            for it in range(MAX_ITER):
                m_prev = ms[it - 1] if it > 0 else 0.0
                s0 = float(1.0 + m_prev)
                s1 = float(-m_prev)
                cur, nxt = it % 2, (it + 1) % 2
                xc_h, xc_t = xH[cur], xT[cur]
                xn_h, xn_t = xH[nxt], xT[nxt]
                last = it == MAX_ITER - 1

                for g in range(NGRP):
                    gs = slice(g * GRP, (g + 1) * GRP)
                    pc = slice(g * HB, (g + 1) * HB)
                    csA = slice(g * GRP, g * GRP + HB)
                    csB = slice(g * GRP + HB, (g + 1) * GRP)

                    wh = psH.tile([KH, GRP], mybir.dt.float32, tag="wh",
                                  name="wh")
                    wt = psT.tile([XT_P, HB], mybir.dt.float32, tag="wt",
                                  name="wt")

                    # half A: contraction block 2 at partition base 0; tail
                    # out-block 64 wide (43 real + 21 zero) -> pads written
                    mm(wh[:, 0:HB], l1a[:], xc_h[:, csA], True, False)
                    mm(wh[:, 0:HB], l2a[0:KT, :], xc_t[0:KT, pc], False, True)
                    mm(wt[0:TB, :], l1b[:], xc_h[:, csA], True, False)
                    mm(wt[0:TB, :], l2b[0:KT, :], xc_t[0:KT, pc], False, True)
                    # half B: tail lands at PSUM partition base 64
                    mm(wh[:, HB:GRP], l1a[:], xc_h[:, csB], True, False)
                    mm(wh[:, HB:GRP], l2a[TB:TB + KT, :],
                       xc_t[TB:TB + KT, pc], False, True)
                    mm(wt[TB:TB + KT, :], l1b[:, 0:KT], xc_h[:, csB],
                       True, False)
                    mm(wt[TB:TB + KT, :], l2b[TB:TB + KT, 0:KT],
                       xc_t[TB:TB + KT, pc], False, True)

                    # evacuate raw u to SBUF (frees the PSUM tile and gives
                    # the fused op an all-SBUF read path); on the final
                    # group the tail goes first (ACT is in-order and the
                    # tail fuse+DMA chain ends the program)
                    if last and g == NGRP - 1:
                        nc.scalar.copy(esT[cur][:, pc], wt[:])
                        nc.scalar.copy(esH[cur][:, gs], wh[:])
                    else:
                        nc.scalar.copy(esH[cur][:, gs], wh[:])
                        nc.scalar.copy(esT[cur][:, pc], wt[:])

                    # fused momentum + soft-threshold over PAIRS of groups
                    # (g odd merges g-1 and g: one op per two groups saves
                    # the per-op access overhead); group 4 runs solo.  The
                    # final iteration keeps per-group ops for DMA overlap.
                    if last:
                        if g == NGRP - 1:
                            # the end-of-program chain: run the small tail
                            # unit first and split the head DMA so the
                            # last-landing transfer is short
                            nc.vector._custom_dve(fused_op, out=foT[:, pc],
                                                  in0=esT[cur][:, pc],
                                                  in1=esT[nxt][:, pc],
                                                  s0=s0, s1=s1,
                                                  imm2=float(lam))
                            nc.sync.dma_start(out[KH:K, csA], foT[0:KT, pc])
                            nc.sync.dma_start(out[KH:K, csB],
                                              foT[TB:TB + KT, pc])
                            nc.vector._custom_dve(fused_op, out=foH[:, gs],
                                                  in0=esH[cur][:, gs],
                                                  in1=esH[nxt][:, gs],
                                                  s0=s0, s1=s1,
                                                  imm2=float(lam))
                            nc.sync.dma_start(out[0:KH, csA], foH[:, csA])
                            nc.sync.dma_start(out[0:KH, csB], foH[:, csB])
                            continue
                        nc.vector._custom_dve(fused_op, out=foH[:, gs],
                                              in0=esH[cur][:, gs],
                                              in1=esH[nxt][:, gs],
                                              s0=s0, s1=s1, imm2=float(lam))
                        nc.vector._custom_dve(fused_op, out=foT[:, pc],
                                              in0=esT[cur][:, pc],
                                              in1=esT[nxt][:, pc],
                                              s0=s0, s1=s1, imm2=float(lam))
                        nc.sync.dma_start(out[0:KH, gs], foH[:, gs])
                        nc.sync.dma_start(out[KH:K, csA], foT[0:KT, pc])
                        nc.sync.dma_start(out[KH:K, csB],
                                          foT[TB:TB + KT, pc])
                        continue
                    if g in (0, 2):
                        continue          # fused with the next group
                    if g in (1, 3):
                        mgs = slice((g - 1) * GRP, (g + 1) * GRP)
                        mpc = slice((g - 1) * HB, (g + 1) * HB)
                    else:
                        mgs, mpc = gs, pc
                    ihp = esH[nxt][:, mgs] if it > 0 else zst[0:KH, mgs]
                    itp = (esT[nxt][:, mpc] if it > 0
                           else zst[0:XT_P, mpc])
                    nc.vector._custom_dve(fused_op, out=xn_h[0:KH, mgs],
                                          in0=esH[cur][:, mgs],
                                          in1=ihp,
                                          s0=s0, s1=s1, imm2=float(lam))
                    nc.vector._custom_dve(fused_op, out=xn_t[:, mpc],
                                          in0=esT[cur][:, mpc],
                                          in1=itp,
                                          s0=s0, s1=s1, imm2=float(lam))
    nc.finalize()
    return nc


def _get_program(lam, ms):
    key = (round(lam, 12), tuple(round(m, 9) for m in ms))
    if _cache.get("key") != key:
        _cache["consts_meta"] = (lam, ms)
        _cache["nc"] = _build_program()
        _cache["key"] = key
    return _cache["nc"]


# ------------------------------------------------------------------- kernel
def kernel(x, Drr, Dtheta):
    from concourse.bass_utils import run_bass_kernel_spmd

    w1, w2, lam, ms = _host_constants(Drr, Dtheta)
    nc = _get_program(lam, ms)

    W2 = KH + TB
    wts = np.zeros((128, 2 * W2), np.float32)
    wts[:, 0:KH] = w1[:, 0:KH]                  # l1a
    wts[:, KH:KH + KT] = w1[:, KH:K]            # l1b (cols KH+KT..W2 zero)
    wts[0:KT, W2:W2 + KH] = w2[:, 0:KH]         # l2a
    wts[0:KT, W2 + KH:W2 + KH + KT] = w2[:, KH:K]  # l2b (+21 zero cols)

    xc = np.ascontiguousarray(
        np.transpose(x.astype(np.float32), (1, 0, 2)).reshape(T, B * P))

    in_maps = []
    for c in range(NCORES):
        in_maps.append({
            "ycols": np.ascontiguousarray(xc[:, c * NCOLS:(c + 1) * NCOLS]),
            "wts": wts,
        })

    res = run_bass_kernel_spmd(nc, in_maps, core_ids=list(range(NCORES)))
    _cache["last_res"] = res
    full = np.concatenate([r["out"] for r in res.results], axis=1)  # [K, B*P]
    return np.ascontiguousarray(
        full.reshape(K, B, P).transpose(1, 0, 2)).astype(np.float32)


if __name__ == "__main__":
    x = np.random.randn(B, T, P).astype(np.float32)
    Drr = np.random.rand(N_POLES).astype(np.float32)
    Dtheta = np.random.rand(N_POLES).astype(np.float32)
    o = kernel(x, Drr, Dtheta)
    print(o.shape, o.dtype)


# revision 38
# speedup vs baseline: 1.0023x; 1.0023x over previous
"""FISTA sparse-coding encoder kernel for Trainium2 (8 NeuronCores).

Problem: x [2,10,20480] f32, Drr/Dtheta [40] f32.
  D = normalized dictionary [10, 161]
  A = I - D^T D / L,  DtY = D^T Y / L,  lam = gamma / L
  40 FISTA iterations: xn = softshrink(A @ y + DtY); y = xn + m (xn - x_old)
  output sparsecode [2, 161, 20480].

Design ("u-form", v4: fp16 matmuls + PSUM-packed tails):
  - Data-parallel over columns: Y reshaped to [10, 40960]; 5120 columns/core.
  - Momentum identity: A y_i + DtY = (1+m) u_i - m u_{i-1} with
    u_i = A x_i + DtY, so each iteration needs one matmul pass over x_i and
    one fused elementwise op  xn = shrink(s0*u_i - m*u_{i-1}, lam).
  - Output rows split 118 (head) + 43 (tail).  Contraction split 128 + 43:
    block 1 = [x rows 0..117 ; Y glue rows] (DtY via the glue), block 2 =
    x rows 118..160 packed at partition bases 0/64 of xT.
  - Tail packing: the two 512-column halves of a 1024-column group write
    their 43 tail rows into ONE [107, 512] PSUM tile at partition offsets 0
    and 64 (PE quad-tile col positions; half A's tail out-block is 64 wide
    with 21 zero weight columns so the pad partitions are initialized).
    The tail DVE/ACT ops then process 1024 columns in 512 free-cycles,
    cutting elementwise busy/iter from 2N to 1.5N.  PSUM partition offsets
    require 16-bit matmul operands (walrus rejects f32r quad-tiling), so
    x-state and weights are fp16: u stays f32 in PSUM, the momentum
    combines f32 u's, and only x is rounded; measured end-to-end error
    ~9e-3 vs the f32 reference (tolerance 2e-2).
  - Per iteration per group:
      PE:  8 matmuls -> u head [118,1024] + u tail packed [107,512] (f32)
      ACT: evacuate raw u -> esH/esT[it%2] (doubles as next iteration's
           u_{i-1}; all-SBUF reads are cheaper for the DVE than PSUM)
      DVE: fused shrink+momentum  xn = shrink(s0*es_cur + s1*es_prev)
           -> fp16 x state (f32 tiles on the last iteration for the DMA)
  - Warm-up: dummy matmuls ramp the PE p-state before real work dispatches;
    a no-op activation preloads the ACT function table; init is chunked by
    column group so the pipeline starts as soon as group 0 is staged.
"""

import numpy as np

# ---------------------------------------------------------------- constants
B, T, N_POLES, P = 2, 10, 40, 20480
MAX_ITER = 40
GAMMA = 0.01
K = 4 * N_POLES + 1          # 161
NCORES = 8
NCOLS = B * P // NCORES      # 5120 columns per core
GRP = 1024                   # column group (head psum = 2 banks)
HB = 512                     # half-block (one PSUM bank, matmul free dim)
NGRP = NCOLS // GRP          # 5
KH = 118                     # head out rows (x rows 0..117)
KT = K - KH                  # 43 tail out rows (x rows 118..160)
PCOLS = NCOLS // 2           # 2560 packed tail columns
TB = 64                      # packed-tail half B base partition
XT_P = TB + KT               # 107 partitions of packed tail state

_cache = {}


# ------------------------------------------------------------- custom DVE op
def _register_shrinkmom3():
    """out = relu(w - C2) + min(w + C2, 0)  with  w = in0*s0 + in1*s1.

    Softshrink of an affine combination of two raw tensors:
    s0 = (1+m_prev), s1 = -m_prev, imm2 = lam.  8 ALU stages.
    """
    import concourse.dve_ops as dve_ops
    from concourse.dve_spec import (
        Spec, Src0, Src1, C0, C1, C2, Zero, relu, minn, lower,
    )
    from concourse.dve_spec import _has_src1 as has_src1
    from concourse.dve_uop import DveOpSpec

    name = "ANT_SHRINKMOM3_FISTA"
    if any(op.name == name for op in dve_ops.OPS):
        return next(op for op in dve_ops.OPS if op.name == name)

    w = Src0 * C0 + Src1 * C1
    spec = Spec(
        body=relu(w - C2) + minn(w + C2, Zero),
        reference=lambda in0, in1, s0=1.0, s1=0.0, imm2=0.0: (
            lambda ww: (np.maximum(ww - imm2, 0.0)
                        + np.minimum(ww + imm2, 0.0)).astype(np.float32)
        )(in0 * s0 + in1 * s1),
    )
    op = dve_ops.DveOp(name, spec, subdim=False, uops_sha={})
    dve_ops.OPS.append(op)
    dve_ops.CUSTOM_DVE_SPECS[name] = spec
    dve_ops._SUB_OPCODE_FOR_NAME[name] = (
        dve_ops._CUSTOM_DVE_ROW_BASE + len(dve_ops.OPS) - 1
    )
    for ver in ("v3", "v4"):
        compiled = DveOpSpec(
            name=name,
            opcode=dve_ops.get_dve_sub_opcode(name),
            uops=lower(spec, ver=ver),
            rd1_en=has_src1(spec),
        )
        op.uops_sha[ver] = compiled.sha(ver)
    return op


# ------------------------------------------------------------ host constants
def _host_constants(Drr, Dtheta):
    r = Drr.astype(np.float64)
    th = Dtheta.astype(np.float64)
    i = np.arange(T, dtype=np.float64)[:, None]
    pr = r[None, :] ** i
    sgn = np.where(np.arange(T)[:, None] % 2 == 0, 1.0, -1.0)
    c = np.cos(i * th[None, :])
    s = np.sin(i * th[None, :])
    ones = np.ones((T, 1))
    dic = np.concatenate([ones, pr * c, sgn * pr * c, pr * s, sgn * pr * s], axis=1)
    G = np.linalg.norm(dic, axis=0)
    G = np.where(G == 0, np.sqrt(float(T)), G)
    D = (dic / G).astype(np.float32)            # [T, K]

    D64 = D.astype(np.float64)
    DtD = D64.T @ D64
    L = float(np.linalg.norm(DtD))              # Frobenius
    A = np.eye(K) - DtD / L                     # [K, K] (symmetric)
    lam = float(GAMMA / L)

    # contraction block 1 rows: [A rows 0..117 ; D/L glue rows]  [128, K]
    w1 = np.concatenate([A[0:KH, :], D64 / L], axis=0).astype(np.float32)
    # contraction block 2 rows: A rows 118..160  [43, K]
    w2 = A[KH:K, :].astype(np.float32)

    # momentum coefficients m_i = (t_i - 1)/t_{i+1}, t_0 = 1
    ms = []
    t = 1.0
    for _ in range(MAX_ITER):
        t_new = (1.0 + np.sqrt(1.0 + 4.0 * t * t)) / 2.0
        ms.append((t - 1.0) / t_new)
        t = t_new
    return w1, w2, lam, ms


# ------------------------------------------------------------- bass program
def _build_program():
    import concourse.mybir as mybir
    import concourse.tile as tile
    from concourse import bacc

    fused_op = _register_shrinkmom3()

    f32 = mybir.dt.float32
    f16 = mybir.dt.float16

    nc = bacc.Bacc("TRN2", target_bir_lowering=False, debug=False,
                   num_devices=NCORES)

    ycols = nc.dram_tensor("ycols", [T, NCOLS], f32, kind="ExternalInput")
    # all weights in one tensor (one DMA): cols 0:118 = l1a, 118:182 = l1b,
    # 182:300 = l2a (rows 0:43), 300:364 = l2b (rows 0:43).  The tail-out
    # weights (l1b/l2b) carry 21 zero columns so the matmuls also zero-fill
    # the pad partitions 43..63 of the packed PSUM tile (free — matmul cost
    # depends only on the moving free size).
    WCOLS = 2 * (KH + TB)
    d_wts = nc.dram_tensor("wts", [128, WCOLS], f32, kind="ExternalInput")
    out = nc.dram_tensor("out", [K, NCOLS], f32, kind="ExternalOutput")

    lam, ms = _cache["consts_meta"]

    with tile.TileContext(nc) as tc:
        with (
            tc.tile_pool(name="state", bufs=1) as st,
            tc.tile_pool(name="wts", bufs=1) as wts,
            tc.tile_pool(name="psH", bufs=2, space="PSUM") as psH,
            tc.tile_pool(name="psT", bufs=3, space="PSUM") as psT,
            tc.tile_pool(name="psW", bufs=1, space="PSUM") as psW,
        ):
            # ---- persistent state -------------------------------------
            # xH rows 0..117 = x head; rows 118..127 = Y glue (written once)
            xH = [st.tile([128, NCOLS], f16, tag=f"xH{b}", name=f"xH{b}")
                  for b in range(2)]
            # packed tail: half A at partitions 0..42, half B at 64..106
            xT = [st.tile([XT_P, PCOLS], f16, tag=f"xT{b}", name=f"xT{b}")
                  for b in range(2)]
            # raw u evacuations, double-buffered: esX[it%2] holds u_it; the
            # fused op reads the current one as in0 (all-SBUF is cheaper
            # for the DVE than PSUM) and the previous one as in1
            esH = [st.tile([KH, NCOLS], f32, tag=f"esH{b}", name=f"esH{b}")
                   for b in range(2)]
            esT = [st.tile([XT_P, PCOLS], f32, tag=f"esT{b}", name=f"esT{b}")
                   for b in range(2)]
            # f32 copies of the final iterate for the output DMA
            foH = st.tile([KH, NCOLS], f32, tag="foH", name="foH")
            foT = st.tile([XT_P, PCOLS], f32, tag="foT", name="foT")

            # fp32 staging for DMA'd weights -> fp16 copies (on DVE, which
            # is idle during init; ACT is busy with the xH[0] chunks)
            ws = wts.tile([128, WCOLS], f32, tag="ws", name="ws")
            l1a = wts.tile([128, KH], f16, tag="l1a", name="l1a")
            l1b = wts.tile([128, TB], f16, tag="l1b", name="l1b")
            # block-2 weights duplicated at partition bases 0 and 64
            l2a = wts.tile([XT_P, KH], f16, tag="l2a", name="l2a")
            l2b = wts.tile([XT_P, TB], f16, tag="l2b", name="l2b")

            # ACT warm-up: a no-op activation so the one-time activation
            # table load (1.3us) happens immediately instead of on the
            # critical first xH copy
            tiny = wts.tile([8, 16], f32, tag="tiny", name="tiny")
            nc.gpsimd.memset(tiny[:], 0.0)
            nc.scalar.mul(tiny[:], tiny[:], 1.0)

            # PE warm-up: the cost model prices matmuls at dispatch time by
            # how long the PE has been continuously busy (p-state ramp), so
            # a short chain of dummy matmuls on an early-ready zero tile
            # ramps the clock before the real work dispatches
            warm = wts.tile([64, HB], f16, tag="warm", name="warm")
            wps = psW.tile([64, HB], mybir.dt.float32, tag="wps", name="wps")
            nc.gpsimd.memset(warm[:], 0.0)
            for i in range(7):
                nc.tensor.matmul(wps[:], warm[:, 0:64], warm[:],
                                 start=(i == 0), stop=(i == 6),
                                 skip_group_check=True)

            nc.sync.dma_start(ws[:], d_wts[:])
            W2 = KH + TB
            nc.vector.tensor_copy(l1a[:], ws[:, 0:KH])
            nc.vector.tensor_copy(l1b[:], ws[:, KH:W2])
            nc.vector.tensor_copy(l2a[0:KT, :], ws[0:KT, W2:W2 + KH])
            nc.vector.tensor_copy(l2b[0:KT, :], ws[0:KT, W2 + KH:WCOLS])
            nc.vector.tensor_copy(l2a[TB:TB + KT, :], ws[0:KT, W2:W2 + KH])
            nc.vector.tensor_copy(l2b[TB:TB + KT, :], ws[0:KT, W2 + KH:WCOLS])

            # ---- init ------------------------------------------------
            # x_0 = 0 (so iteration 0 computes u_0 = DtY from the glue
            # alone).  Zeros + glue-Y staged in f32 and engine-copied
            # (converting) into the fp16 state.  Everything is chunked by
            # column group so iteration 0's first matmuls start as soon as
            # group 0's state is ready.  Iteration 0's fused ops read their
            # (x0-scaled, s1=0) u_{i-1} term from the zeroed zst staging,
            # so the es tiles need no init at all.
            zst = wts.tile([128, NCOLS], f32, tag="zst", name="zst")
            # 3 input DMAs: a small first chunk so group 0 unblocks fast
            ysl = [slice(0, GRP), slice(GRP, 3 * GRP), slice(3 * GRP, NCOLS)]
            for g in range(NGRP):
                gs = slice(g * GRP, (g + 1) * GRP)
                pc = slice(g * HB, (g + 1) * HB)
                nc.gpsimd.memset(zst[0:KH, gs], 0.0)
                if g == 0:
                    nc.sync.dma_start(zst[KH:128, ysl[0]], ycols[:, ysl[0]])
                elif g == 1:
                    nc.sync.dma_start(zst[KH:128, ysl[1]], ycols[:, ysl[1]])
                elif g == 3:
                    nc.sync.dma_start(zst[KH:128, ysl[2]], ycols[:, ysl[2]])
                if g == 0:
                    # split so half A's matmuls unblock ~0.6us earlier
                    nc.scalar.copy(xH[0][:, 0:HB], zst[:, 0:HB])
                    nc.scalar.copy(xH[0][:, HB:GRP], zst[:, HB:GRP])
                else:
                    nc.scalar.copy(xH[0][:, gs], zst[:, gs])
                nc.gpsimd.tensor_copy(xT[0][:, pc],
                                      zst[0:XT_P, g * HB:(g + 1) * HB])
                # xH[1] only needs its glue rows before iteration 1 reads
                # them (rows 0..117 are written by iteration 0's DVE)
                nc.gpsimd.tensor_copy(xH[1][96:128, gs], zst[96:128, gs])

            def mm(ps, lhsT, rhs, start, stop):
                nc.tensor.matmul(ps, lhsT, rhs, start=start, stop=stop,
                                 skip_group_check=True)

            for it in range(MAX_ITER):
                m_prev = ms[it - 1] if it > 0 else 0.0
                s0 = float(1.0 + m_prev)
                s1 = float(-m_prev)
                cur, nxt = it % 2, (it + 1) % 2
                xc_h, xc_t = xH[cur], xT[cur]
                xn_h, xn_t = xH[nxt], xT[nxt]
                last = it == MAX_ITER - 1

                for g in range(NGRP):
                    gs = slice(g * GRP, (g + 1) * GRP)
                    pc = slice(g * HB, (g + 1) * HB)
                    csA = slice(g * GRP, g * GRP + HB)
                    csB = slice(g * GRP + HB, (g + 1) * GRP)

                    wh = psH.tile([KH, GRP], mybir.dt.float32, tag="wh",
                                  name="wh")
                    wt = psT.tile([XT_P, HB], mybir.dt.float32, tag="wt",
                                  name="wt")

                    # half A: contraction block 2 at partition base 0; tail
                    # out-block 64 wide (43 real + 21 zero) -> pads written
                    mm(wh[:, 0:HB], l1a[:], xc_h[:, csA], True, False)
                    mm(wh[:, 0:HB], l2a[0:KT, :], xc_t[0:KT, pc], False, True)
                    mm(wt[0:TB, :], l1b[:], xc_h[:, csA], True, False)
                    mm(wt[0:TB, :], l2b[0:KT, :], xc_t[0:KT, pc], False, True)
                    # half B: tail lands at PSUM partition base 64
                    mm(wh[:, HB:GRP], l1a[:], xc_h[:, csB], True, False)
                    mm(wh[:, HB:GRP], l2a[TB:TB + KT, :],
                       xc_t[TB:TB + KT, pc], False, True)
                    mm(wt[TB:TB + KT, :], l1b[:, 0:KT], xc_h[:, csB],
                       True, False)
                    mm(wt[TB:TB + KT, :], l2b[TB:TB + KT, 0:KT],
                       xc_t[TB:TB + KT, pc], False, True)

                    # evacuate raw u to SBUF (frees the PSUM tile and gives
                    # the fused op an all-SBUF read path)
                    nc.scalar.copy(esH[cur][:, gs], wh[:])
                    nc.scalar.copy(esT[cur][:, pc], wt[:])

                    # fused momentum + soft-threshold over PAIRS of groups
                    # (g odd merges g-1 and g: one op per two groups saves
                    # the per-op access overhead); group 4 runs solo.  The
                    # final iteration keeps per-group ops for DMA overlap.
                    if last:
                        nc.vector._custom_dve(fused_op, out=foH[:, gs],
                                              in0=esH[cur][:, gs],
                                              in1=esH[nxt][:, gs],
                                              s0=s0, s1=s1, imm2=float(lam))
                        nc.vector._custom_dve(fused_op, out=foT[:, pc],
                                              in0=esT[cur][:, pc],
                                              in1=esT[nxt][:, pc],
                                              s0=s0, s1=s1, imm2=float(lam))
                        nc.sync.dma_start(out[0:KH, gs], foH[:, gs])
                        nc.sync.dma_start(out[KH:K, csA], foT[0:KT, pc])
                        nc.sync.dma_start(out[KH:K, csB],
                                          foT[TB:TB + KT, pc])
                        continue
                    if g in (0, 2):
                        continue          # fused with the next group
                    if g in (1, 3):
                        mgs = slice((g - 1) * GRP, (g + 1) * GRP)
                        mpc = slice((g - 1) * HB, (g + 1) * HB)
                    else:
                        mgs, mpc = gs, pc
                    ihp = esH[nxt][:, mgs] if it > 0 else zst[0:KH, mgs]
                    itp = (esT[nxt][:, mpc] if it > 0
                           else zst[0:XT_P, mpc])
                    nc.vector._custom_dve(fused_op, out=xn_h[0:KH, mgs],
                                          in0=esH[cur][:, mgs],
                                          in1=ihp,
                                          s0=s0, s1=s1, imm2=float(lam))
                    nc.vector._custom_dve(fused_op, out=xn_t[:, mpc],
                                          in0=esT[cur][:, mpc],
                                          in1=itp,
                                          s0=s0, s1=s1, imm2=float(lam))
    nc.finalize()
    return nc


def _get_program(lam, ms):
    key = (round(lam, 12), tuple(round(m, 9) for m in ms))
    if _cache.get("key") != key:
        _cache["consts_meta"] = (lam, ms)
        _cache["nc"] = _build_program()
        _cache["key"] = key
    return _cache["nc"]


# ------------------------------------------------------------------- kernel
def kernel(x, Drr, Dtheta):
    from concourse.bass_utils import run_bass_kernel_spmd

    w1, w2, lam, ms = _host_constants(Drr, Dtheta)
    nc = _get_program(lam, ms)

    W2 = KH + TB
    wts = np.zeros((128, 2 * W2), np.float32)
    wts[:, 0:KH] = w1[:, 0:KH]                  # l1a
    wts[:, KH:KH + KT] = w1[:, KH:K]            # l1b (cols KH+KT..W2 zero)
    wts[0:KT, W2:W2 + KH] = w2[:, 0:KH]         # l2a
    wts[0:KT, W2 + KH:W2 + KH + KT] = w2[:, KH:K]  # l2b (+21 zero cols)

    xc = np.ascontiguousarray(
        np.transpose(x.astype(np.float32), (1, 0, 2)).reshape(T, B * P))

    in_maps = []
    for c in range(NCORES):
        in_maps.append({
            "ycols": np.ascontiguousarray(xc[:, c * NCOLS:(c + 1) * NCOLS]),
            "wts": wts,
        })

    res = run_bass_kernel_spmd(nc, in_maps, core_ids=list(range(NCORES)))
    _cache["last_res"] = res
    full = np.concatenate([r["out"] for r in res.results], axis=1)  # [K, B*P]
    return np.ascontiguousarray(
        full.reshape(K, B, P).transpose(1, 0, 2)).astype(np.float32)


if __name__ == "__main__":
    x = np.random.randn(B, T, P).astype(np.float32)
    Drr = np.random.rand(N_POLES).astype(np.float32)
    Dtheta = np.random.rand(N_POLES).astype(np.float32)
    o = kernel(x, Drr, Dtheta)
    print(o.shape, o.dtype)
